# revision 7
# baseline (speedup 1.0000x reference)
"""ContextualLoss forward on 8 trn2 NeuronCores.

Problem: X, Y [4, 256, 64, 64] f32 ->  loss [4] f32
  y_mean[c] = mean_hw(Y);  Xc = X - y_mean; Yc = Y - y_mean
  Xn, Yn: L2-normalized over C per spatial position; S = Xn^T @ Yn  [N, N], N=4096
  d = 1 - S; dmin = row min d; w = exp((1 - d/(dmin+1e-3))/0.1); A = w/rowsum(w)
  loss_b = -log(mean_n max_m A[n, m])

Key algebra (per row n), with s = Xc^T @ Yn (X centered but unnormalized,
g = 1/||Xc||, t = s*g the cosine, smax = row max s):
  max_m A[n,:] = 1 / sum_m exp(a_n*(s_nm - smax_n)),
  a_n = 10*g_n/(1.001 - smax_n*g_n)
The exp bias cancels between numerator and denominator and the argument is
always <= 0 (overflow-safe since pass A and pass B compute bitwise-equal
matmuls), so no wmax term is needed.

Structure per 128-row block: pass A matmul -> PSUM -> VectorE row max;
then per-row scale a / bias -a*smax; pass B matmul -> PSUM -> ScalarE
Exp with accum_out giving Z.  Blocks are software-pipelined depth-2
(PE order A0 A1 A2 B0 A3 B1 ...) so the max/scale chain of block nb hides
under pass-A matmuls of blocks nb+1/nb+2 and the PE never stalls.

Sharding: 8 cores = 4 batch samples x 2 row-halves of 2048 rows each.
Host: loss_b = -log((core0.acc.sum + core1.acc.sum)/4096) where
acc[p] = sum_nb 1/Z.
"""

import numpy as np

B, C, HW = 4, 256, 4096
HALF = HW // 2
NCORES = 8
NB = HALF // 128      # 16 row blocks per core
MT = HW // 1024       # 4 psum tiles of [128,1024] per block
H_INV = 10.0          # 1/h with h = 0.1

_nc_cache = None


def _build():
    import concourse.bass as bass
    import concourse.bacc as bacc
    import concourse.tile as tile
    from concourse import mybir

    f32 = mybir.dt.float32
    bf16 = mybir.dt.bfloat16
    AF = mybir.ActivationFunctionType
    OP = mybir.AluOpType
    AX = mybir.AxisListType

    nc = bacc.Bacc(None)

    y_dram = nc.dram_tensor("y", [C, HW], f32, kind="ExternalInput")
    x_dram = nc.dram_tensor("xh", [C, HALF], f32, kind="ExternalInput")
    out_dram = nc.dram_tensor("out", [128, 1], f32, kind="ExternalOutput")
    xt_dram = nc.dram_tensor("xt_scratch", [1, HALF], f32)  # transpose bounce

    with tile.TileContext(nc) as tc:
        with (
            tc.tile_pool(name="big", bufs=1) as big,
            tc.tile_pool(name="singles", bufs=1) as singles,
            tc.tile_pool(name="rows", bufs=1) as rows,
            tc.tile_pool(name="stats", bufs=4) as stats,
            tc.tile_pool(name="dumps", bufs=2) as dumps,
        ):
            # ---------------- constants ----------------
            ones_col = singles.tile([128, 1], bf16)
            nc.vector.memset(ones_col, 1.0)
            cm1 = singles.tile([128, 1], f32)
            nc.vector.memset(cm1, -1.0)
            cm1p001 = singles.tile([128, 1], f32)
            nc.vector.memset(cm1p001, -1.001)

            # ---------------- load inputs ----------------
            y_sb = [big.tile([128, HW], f32, tag=f"y{i}", name=f"y{i}") for i in range(2)]
            x_sb = [big.tile([128, HALF], f32, tag=f"x{i}", name=f"x{i}") for i in range(2)]
            for i in range(2):
                nc.sync.dma_start(out=y_sb[i], in_=y_dram[128 * i : 128 * (i + 1), :])
            for i in range(2):
                nc.sync.dma_start(out=x_sb[i], in_=x_dram[128 * i : 128 * (i + 1), :])

            # ---------------- spatial mean of Y over positions ----------------
            yn = [big.tile([128, HW], bf16, tag=f"yn{i}", name=f"yn{i}") for i in range(2)]
            ysum = [singles.tile([128, 1], f32, tag=f"ysum{i}", name=f"ysum{i}") for i in range(2)]
            # tile 0 summed on DVE; tile 1 on ScalarE (accum of identity copy
            # into yn[1], used purely as scratch here and overwritten later)
            nc.vector.reduce_sum(out=ysum[0], in_=y_sb[0], axis=AX.X)
            nc.scalar.activation(
                out=yn[1], in_=y_sb[1], func=AF.Identity, bias=0.0, scale=1.0,
                accum_out=ysum[1],
            )
            negmean = [singles.tile([128, 1], f32, tag=f"nm{i}", name=f"nm{i}") for i in range(2)]
            for i in range(2):
                nc.vector.tensor_scalar_mul(out=negmean[i], in0=ysum[i], scalar1=-1.0 / HW)

            # squares of centered X/Y in bf16 (feed the sum-of-squares matmuls)
            ysq = [big.tile([128, HW], bf16, tag=f"ysq{i}", name=f"ysq{i}") for i in range(2)]
            xsq = [big.tile([128, HALF], bf16, tag=f"xsq{i}", name=f"xsq{i}") for i in range(2)]
            for i in range(2):
                nc.scalar.activation(
                    out=ysq[i], in_=y_sb[i], func=AF.Square, bias=negmean[i], scale=1.0
                )
            for i in range(2):
                nc.scalar.activation(
                    out=xsq[i], in_=x_sb[i], func=AF.Square, bias=negmean[i], scale=1.0
                )

            # centered X in bf16 (matmul lhsT)
            xcb = [big.tile([128, HALF], bf16, tag=f"xcb{i}", name=f"xcb{i}") for i in range(2)]
            for i in range(2):
                nc.scalar.activation(
                    out=xcb[i], in_=x_sb[i], func=AF.Identity, bias=negmean[i], scale=1.0
                )

            lnx_row = rows.tile([1, HALF], f32)
            lny_row = rows.tile([1, HW], f32)
            invny_row = rows.tile([1, HW], f32)
            lnx_t = singles.tile([128, NB], f32)
            g_t = singles.tile([128, NB], f32)
            gm10 = singles.tile([128, NB], f32)

            # X norms first (the transpose bounce has long latency)
            with tc.tile_pool(name="psx", bufs=1, space="PSUM") as psx:
                ssx = psx.tile([1, HALF], f32)
                for t in range(HALF // 512):
                    sl = slice(t * 512, (t + 1) * 512)
                    for i in range(2):
                        nc.tensor.matmul(
                            ssx[0:1, sl], ones_col, xsq[i][:, sl],
                            start=(i == 0), stop=(i == 1),
                        )
                for t in range(HALF // 1024):
                    sl = slice(t * 1024, (t + 1) * 1024)
                    nc.scalar.activation(
                        out=lnx_row[0:1, sl], in_=ssx[0:1, sl], func=AF.Ln,
                        bias=0.0, scale=1.0,
                    )
            # bounce ln(ssx) [1,2048] -> [128,16]; then g = exp(-0.5 ln) per row
            nc.gpsimd.dma_start(out=xt_dram[:, :], in_=lnx_row)
            nc.gpsimd.dma_start(
                out=lnx_t, in_=xt_dram.rearrange("o (j p) -> (o p) j", p=128)
            )
            nc.scalar.activation(out=g_t, in_=lnx_t, func=AF.Exp, bias=0.0, scale=-0.5)
            nc.vector.tensor_scalar_mul(out=gm10, in0=g_t, scalar1=-H_INV)

            # Y norms: ss -> ln -> exp(-0.5) = 1/||Yc||
            with tc.tile_pool(name="psy", bufs=1, space="PSUM") as psy:
                ssy = psy.tile([1, HW], f32)
                for t in range(HW // 512):
                    sl = slice(t * 512, (t + 1) * 512)
                    for i in range(2):
                        nc.tensor.matmul(
                            ssy[0:1, sl], ones_col, ysq[i][:, sl],
                            start=(i == 0), stop=(i == 1),
                        )
                for t in range(HW // 1024):
                    sl = slice(t * 1024, (t + 1) * 1024)
                    nc.scalar.activation(
                        out=lny_row[0:1, sl], in_=ssy[0:1, sl], func=AF.Ln,
                        bias=0.0, scale=1.0,
                    )
            nc.scalar.activation(
                out=invny_row, in_=lny_row, func=AF.Exp, bias=0.0, scale=-0.5
            )

            # broadcast 1/||Yc|| across partitions, then Yn = (Y - mean)*invnY
            invny_b = big.tile([128, HW], f32, tag="invny_b")
            for chunk in range(HW // 512):
                sl = slice(chunk * 512, (chunk + 1) * 512)
                nc.gpsimd.partition_broadcast(invny_b[:, sl], invny_row[0:1, sl])
            for i in range(2):
                nc.vector.scalar_tensor_tensor(
                    out=yn[i], in0=y_sb[i], scalar=negmean[i], in1=invny_b,
                    op0=OP.add, op1=OP.mult,
                )

            # -------- main loop: depth-2 software-pipelined blocks --------
            zall = singles.tile([128, NB * MT], f32)
            scale_state = {}

            with (
                tc.tile_pool(name="psA", bufs=2, space="PSUM") as psA,
                tc.tile_pool(name="psB", bufs=2, space="PSUM") as psB,
            ):
                def emit_passA_and_scale(nb):
                    nsl = slice(nb * 128, (nb + 1) * 128)
                    mx4 = stats.tile([128, MT], f32, tag="mx4")
                    for j in range(MT):
                        pa = psA.tile([128, 1024], f32, tag="pa")
                        for jj in range(2):
                            msl = slice(j * 1024 + jj * 512, j * 1024 + (jj + 1) * 512)
                            osl = slice(jj * 512, (jj + 1) * 512)
                            nc.tensor.matmul(
                                pa[:, osl], xcb[0][:, nsl], yn[0][:, msl],
                                start=True, stop=False,
                            )
                            nc.tensor.matmul(
                                pa[:, osl], xcb[1][:, nsl], yn[1][:, msl],
                                start=False, stop=True,
                            )
                        nc.vector.reduce_max(out=mx4[:, j : j + 1], in_=pa, axis=AX.X)
                    smax = stats.tile([128, 1], f32, tag="smax")
                    nc.vector.reduce_max(out=smax, in_=mx4, axis=AX.X)
                    # a = 10*g/(1.001-smax*g) = rr*(-10g), rr = 1/(smax*g-1.001)
                    ndm = stats.tile([128, 1], f32, tag="ndm")
                    nc.scalar.activation(
                        out=ndm, in_=smax, func=AF.Identity,
                        bias=cm1p001, scale=g_t[:, nb : nb + 1],
                    )
                    rr = stats.tile([128, 1], f32, tag="rr")
                    nc.vector.reciprocal(out=rr, in_=ndm)
                    a_col = stats.tile([128, 1], f32, tag="acol")
                    nc.scalar.activation(
                        out=a_col, in_=rr, func=AF.Identity,
                        bias=0.0, scale=gm10[:, nb : nb + 1],
                    )
                    eb = stats.tile([128, 1], f32, tag="eb")
                    nc.vector.scalar_tensor_tensor(
                        out=eb, in0=a_col, scalar=smax, in1=cm1,
                        op0=OP.mult, op1=OP.mult,
                    )
                    scale_state[nb] = (a_col, eb)

                def emit_passB(nb):
                    nsl = slice(nb * 128, (nb + 1) * 128)
                    a_col, eb = scale_state.pop(nb)
                    for j in range(MT):
                        pb = psB.tile([128, 1024], f32, tag="pb")
                        for jj in range(2):
                            msl = slice(j * 1024 + jj * 512, j * 1024 + (jj + 1) * 512)
                            osl = slice(jj * 512, (jj + 1) * 512)
                            nc.tensor.matmul(
                                pb[:, osl], xcb[0][:, nsl], yn[0][:, msl],
                                start=True, stop=False,
                            )
                            nc.tensor.matmul(
                                pb[:, osl], xcb[1][:, nsl], yn[1][:, msl],
                                start=False, stop=True,
                            )
                        dump = dumps.tile([128, 1024], bf16, tag="dump")
                        nc.scalar.activation(
                            out=dump, in_=pb, func=AF.Exp,
                            bias=eb, scale=a_col,
                            accum_out=zall[:, nb * MT + j : nb * MT + j + 1],
                        )

                for nb in range(NB):
                    emit_passA_and_scale(nb)
                    if nb >= 2:
                        emit_passB(nb - 2)
                emit_passB(NB - 2)
                emit_passB(NB - 1)

            # ---------------- epilogue: acc_p = sum_nb 1/Z ----------------
            zs = singles.tile([128, NB], f32)
            nc.vector.reduce_sum(
                out=zs, in_=zall.rearrange("p (nb mt) -> p nb mt", mt=MT), axis=AX.X
            )
            rz = singles.tile([128, NB], f32)
            nc.vector.reciprocal(out=rz, in_=zs)
            acc = singles.tile([128, 1], f32)
            nc.vector.reduce_sum(out=acc, in_=rz, axis=AX.X)
            nc.sync.dma_start(out=out_dram[:, :], in_=acc)

    nc.finalize()
    return nc


def _get_nc():
    global _nc_cache
    if _nc_cache is None:
        _nc_cache = _build()
    return _nc_cache


def run_cores(inputs, **kwargs):
    """Run the 8-core SPMD kernel; returns (loss[4], BassKernelResults)."""
    from concourse.bass_utils import run_bass_kernel_spmd

    nc = _get_nc()
    X = np.asarray(inputs["X_features"], dtype=np.float32).reshape(B, C, HW)
    Y = np.asarray(inputs["Y_features"], dtype=np.float32).reshape(B, C, HW)
    in_maps = []
    for core in range(NCORES):
        b, h = divmod(core, 2)
        in_maps.append(
            {
                "y": np.ascontiguousarray(Y[b]),
                "xh": np.ascontiguousarray(X[b, :, h * HALF : (h + 1) * HALF]),
            }
        )
    res = run_bass_kernel_spmd(nc, in_maps, core_ids=list(range(NCORES)), **kwargs)
    acc = np.stack(
        [res.results[i]["out"].reshape(-1).astype(np.float64) for i in range(NCORES)]
    )  # [8, 128]
    cx = acc.reshape(B, 2 * 128).sum(axis=1) / HW
    loss = (-np.log(cx)).astype(np.float32)
    return loss, res


def kernel(**inputs):
    return run_cores(inputs)[0]


# revision 10
# speedup vs baseline: 1.2232x; 1.2232x over previous
"""ContextualLoss forward on 8 trn2 NeuronCores.

Problem: X, Y [4, 256, 64, 64] f32 ->  loss [4] f32
  y_mean[c] = mean_hw(Y);  Xc = X - y_mean; Yc = Y - y_mean
  Xn, Yn: L2-normalized over C per spatial position; S = Xn^T @ Yn  [N, N], N=4096
  d = 1 - S; dmin = row min d; w = exp((1 - d/(dmin+1e-3))/0.1); A = w/rowsum(w)
  loss_b = -log(mean_n max_m A[n, m])

Key algebra (per row n), with s = Xc^T @ Yn (X centered but unnormalized,
g = 1/||Xc||, t = s*g the cosine, smax = row max s):
  max_m A[n,:] = 1 / sum_m exp(a_n*(s_nm - smax_n)),
  a_n = 10*g_n/(1.001 - smax_n*g_n)
The exp bias cancels between numerator and denominator and the argument is
always <= 0 (overflow-safe since pass A and pass B compute bitwise-equal
matmuls), so no wmax term is needed.

Structure per 128-row block: pass A matmul -> PSUM -> VectorE row max;
then per-row scale a / bias -a*smax; pass B matmul -> PSUM -> ScalarE
Exp with accum_out giving Z.  Blocks are software-pipelined depth-2
(PE order A0 A1 A2 B0 A3 B1 ...) so the max/scale chain of block nb hides
under pass-A matmuls of blocks nb+1/nb+2 and the PE never stalls.

Sharding: 8 cores = 4 batch samples x 2 row-halves of 2048 rows each.
Host: loss_b = -log((core0.acc.sum + core1.acc.sum)/4096) where
acc[p] = sum_nb 1/Z.
"""

import numpy as np

B, C, HW = 4, 256, 4096
HALF = HW // 2
NCORES = 8
NB = HALF // 128      # 16 row blocks per core
MT = HW // 1024       # 4 psum tiles of [128,1024] per block
H_INV = 10.0          # 1/h with h = 0.1

_nc_cache = None


def _build():
    import concourse.bass as bass
    import concourse.bacc as bacc
    import concourse.tile as tile
    from concourse import mybir

    f32 = mybir.dt.float32
    bf16 = mybir.dt.bfloat16
    AF = mybir.ActivationFunctionType
    OP = mybir.AluOpType
    AX = mybir.AxisListType

    nc = bacc.Bacc(None)

    y_dram = nc.dram_tensor("y", [C, HW], f32, kind="ExternalInput")
    x_dram = nc.dram_tensor("xh", [C, HALF], f32, kind="ExternalInput")
    out_dram = nc.dram_tensor("out", [128, 1], f32, kind="ExternalOutput")
    xt_dram = nc.dram_tensor("xt_scratch", [1, HALF], f32)  # transpose bounce

    with tile.TileContext(nc) as tc:
        with (
            tc.tile_pool(name="big", bufs=1) as big,
            tc.tile_pool(name="singles", bufs=1) as singles,
            tc.tile_pool(name="rows", bufs=1) as rows,
            tc.tile_pool(name="stats", bufs=6) as stats,
            tc.tile_pool(name="dumps", bufs=2) as dumps,
        ):
            # ---------------- constants ----------------
            ones_col = singles.tile([128, 1], bf16)
            nc.vector.memset(ones_col, 1.0)
            cm1 = singles.tile([128, 1], f32)
            nc.vector.memset(cm1, -1.0)
            cm1p001 = singles.tile([128, 1], f32)
            nc.vector.memset(cm1p001, -1.001)

            # ---------------- load inputs ----------------
            y_sb = [big.tile([128, HW], f32, tag=f"y{i}", name=f"y{i}") for i in range(2)]
            x_sb = [big.tile([128, HALF], f32, tag=f"x{i}", name=f"x{i}") for i in range(2)]
            for i in range(2):
                nc.sync.dma_start(out=y_sb[i], in_=y_dram[128 * i : 128 * (i + 1), :])
            for i in range(2):
                nc.sync.dma_start(out=x_sb[i], in_=x_dram[128 * i : 128 * (i + 1), :])

            # ---------------- spatial mean of Y over positions ----------------
            yn = [big.tile([128, HW], bf16, tag=f"yn{i}", name=f"yn{i}") for i in range(2)]
            ysum = [singles.tile([128, 1], f32, tag=f"ysum{i}", name=f"ysum{i}") for i in range(2)]
            # tile 0 summed on DVE; tile 1 on ScalarE (accum of identity copy
            # into yn[1], used purely as scratch here and overwritten later)
            nc.vector.reduce_sum(out=ysum[0], in_=y_sb[0], axis=AX.X)
            nc.scalar.activation(
                out=yn[1], in_=y_sb[1], func=AF.Identity, bias=0.0, scale=1.0,
                accum_out=ysum[1],
            )
            negmean = [singles.tile([128, 1], f32, tag=f"nm{i}", name=f"nm{i}") for i in range(2)]
            for i in range(2):
                nc.vector.tensor_scalar_mul(out=negmean[i], in0=ysum[i], scalar1=-1.0 / HW)

            # squares of centered X/Y in bf16 (feed the sum-of-squares matmuls)
            ysq = [big.tile([128, HW], bf16, tag=f"ysq{i}", name=f"ysq{i}") for i in range(2)]
            xsq = [big.tile([128, HALF], bf16, tag=f"xsq{i}", name=f"xsq{i}") for i in range(2)]
            for i in range(2):
                nc.scalar.activation(
                    out=ysq[i], in_=y_sb[i], func=AF.Square, bias=negmean[i], scale=1.0
                )
            for i in range(2):
                nc.scalar.activation(
                    out=xsq[i], in_=x_sb[i], func=AF.Square, bias=negmean[i], scale=1.0
                )

            # centered X in bf16 (matmul lhsT)
            xcb = [big.tile([128, HALF], bf16, tag=f"xcb{i}", name=f"xcb{i}") for i in range(2)]
            for i in range(2):
                nc.scalar.activation(
                    out=xcb[i], in_=x_sb[i], func=AF.Identity, bias=negmean[i], scale=1.0
                )

            lnx_row = rows.tile([1, HALF], f32)
            lny_row = rows.tile([1, HW], f32)
            invny_row = rows.tile([1, HW], f32)
            lnx_t = singles.tile([128, NB], f32)
            g_t = singles.tile([128, NB], f32)
            gm10 = singles.tile([128, NB], f32)

            # X norms first (the transpose bounce has long latency)
            with tc.tile_pool(name="psx", bufs=1, space="PSUM") as psx:
                ssx = psx.tile([1, HALF], f32)
                for t in range(HALF // 512):
                    sl = slice(t * 512, (t + 1) * 512)
                    for i in range(2):
                        nc.tensor.matmul(
                            ssx[0:1, sl], ones_col, xsq[i][:, sl],
                            start=(i == 0), stop=(i == 1),
                        )
                for t in range(HALF // 1024):
                    sl = slice(t * 1024, (t + 1) * 1024)
                    nc.scalar.activation(
                        out=lnx_row[0:1, sl], in_=ssx[0:1, sl], func=AF.Ln,
                        bias=0.0, scale=1.0,
                    )
            # bounce ln(ssx) [1,2048] -> [128,16]; then g = exp(-0.5 ln) per row
            nc.gpsimd.dma_start(out=xt_dram[:, :], in_=lnx_row)
            nc.gpsimd.dma_start(
                out=lnx_t, in_=xt_dram.rearrange("o (j p) -> (o p) j", p=128)
            )
            nc.scalar.activation(out=g_t, in_=lnx_t, func=AF.Exp, bias=0.0, scale=-0.5)
            nc.vector.tensor_scalar_mul(out=gm10, in0=g_t, scalar1=-H_INV)

            # Y norms: ss -> ln -> exp(-0.5) = 1/||Yc||
            with tc.tile_pool(name="psy", bufs=1, space="PSUM") as psy:
                ssy = psy.tile([1, HW], f32)
                for t in range(HW // 512):
                    sl = slice(t * 512, (t + 1) * 512)
                    for i in range(2):
                        nc.tensor.matmul(
                            ssy[0:1, sl], ones_col, ysq[i][:, sl],
                            start=(i == 0), stop=(i == 1),
                        )
                for t in range(HW // 1024):
                    sl = slice(t * 1024, (t + 1) * 1024)
                    nc.scalar.activation(
                        out=lny_row[0:1, sl], in_=ssy[0:1, sl], func=AF.Ln,
                        bias=0.0, scale=1.0,
                    )
            nc.scalar.activation(
                out=invny_row, in_=lny_row, func=AF.Exp, bias=0.0, scale=-0.5
            )

            # broadcast 1/||Yc|| across partitions, then Yn = (Y - mean)*invnY
            invny_b = big.tile([128, HW], f32, tag="invny_b")
            for chunk in range(HW // 512):
                sl = slice(chunk * 512, (chunk + 1) * 512)
                nc.gpsimd.partition_broadcast(invny_b[:, sl], invny_row[0:1, sl])
            for i in range(2):
                nc.vector.scalar_tensor_tensor(
                    out=yn[i], in0=y_sb[i], scalar=negmean[i], in1=invny_b,
                    op0=OP.add, op1=OP.mult,
                )

            # -------- main loop: depth-2 software-pipelined blocks --------
            zall = singles.tile([128, NB * MT], f32)
            scale_state = {}

            with (
                tc.tile_pool(name="psA", bufs=2, space="PSUM") as psA,
                tc.tile_pool(name="psB", bufs=2, space="PSUM") as psB,
            ):
                def emit_passA_and_scale(nb):
                    nsl = slice(nb * 128, (nb + 1) * 128)
                    mx4 = stats.tile([128, MT], f32, tag="mx4")
                    for j in range(MT):
                        pa = psA.tile([128, 1024], f32, tag="pa")
                        for jj in range(2):
                            msl = slice(j * 1024 + jj * 512, j * 1024 + (jj + 1) * 512)
                            osl = slice(jj * 512, (jj + 1) * 512)
                            nc.tensor.matmul(
                                pa[:, osl], xcb[0][:, nsl], yn[0][:, msl],
                                start=True, stop=False,
                            )
                            nc.tensor.matmul(
                                pa[:, osl], xcb[1][:, nsl], yn[1][:, msl],
                                start=False, stop=True,
                            )
                        nc.vector.reduce_max(out=mx4[:, j : j + 1], in_=pa, axis=AX.X)
                    smax = stats.tile([128, 1], f32, tag="smax")
                    nc.vector.reduce_max(out=smax, in_=mx4, axis=AX.X)
                    # a = 10*g/(1.001-smax*g) = rr*(-10g), rr = 1/(smax*g-1.001)
                    # eb = -a*smax = 10*rr*(ndm+1.001) = 10 + 10.01*rr (exact).
                    # Whole chain stays on DVE: every dep is DVE-local, so the
                    # FIFO never head-of-line blocks on another engine.
                    ndm = stats.tile([128, 1], f32, tag="ndm")
                    nc.vector.scalar_tensor_tensor(
                        out=ndm, in0=smax, scalar=g_t[:, nb : nb + 1], in1=cm1p001,
                        op0=OP.mult, op1=OP.add,
                    )
                    rr = stats.tile([128, 1], f32, tag="rr")
                    nc.vector.reciprocal(out=rr, in_=ndm)
                    a_col = stats.tile([128, 1], f32, tag="acol")
                    nc.vector.tensor_tensor(
                        out=a_col, in0=rr, in1=gm10[:, nb : nb + 1], op=OP.mult
                    )
                    eb = stats.tile([128, 1], f32, tag="eb")
                    nc.vector.tensor_scalar(
                        out=eb, in0=rr, scalar1=10.01, scalar2=H_INV,
                        op0=OP.mult, op1=OP.add,
                    )
                    scale_state[nb] = (a_col, eb)

                def emit_passB(nb):
                    nsl = slice(nb * 128, (nb + 1) * 128)
                    a_col, eb = scale_state.pop(nb)
                    for j in range(MT):
                        pb = psB.tile([128, 1024], f32, tag="pb")
                        for jj in range(2):
                            msl = slice(j * 1024 + jj * 512, j * 1024 + (jj + 1) * 512)
                            osl = slice(jj * 512, (jj + 1) * 512)
                            nc.tensor.matmul(
                                pb[:, osl], xcb[0][:, nsl], yn[0][:, msl],
                                start=True, stop=False,
                            )
                            nc.tensor.matmul(
                                pb[:, osl], xcb[1][:, nsl], yn[1][:, msl],
                                start=False, stop=True,
                            )
                        dump = dumps.tile([128, 1024], bf16, tag="dump")
                        nc.scalar.activation(
                            out=dump, in_=pb, func=AF.Exp,
                            bias=eb, scale=a_col,
                            accum_out=zall[:, nb * MT + j : nb * MT + j + 1],
                        )

                for nb in range(NB):
                    emit_passA_and_scale(nb)
                    if nb >= 3:
                        emit_passB(nb - 3)
                for nb in range(NB - 3, NB):
                    emit_passB(nb)

            # ---------------- epilogue: acc_p = sum_nb 1/Z ----------------
            zs = singles.tile([128, NB], f32)
            nc.vector.reduce_sum(
                out=zs, in_=zall.rearrange("p (nb mt) -> p nb mt", mt=MT), axis=AX.X
            )
            rz = singles.tile([128, NB], f32)
            nc.vector.reciprocal(out=rz, in_=zs)
            acc = singles.tile([128, 1], f32)
            nc.vector.reduce_sum(out=acc, in_=rz, axis=AX.X)
            nc.sync.dma_start(out=out_dram[:, :], in_=acc)

    nc.finalize()
    return nc


def _get_nc():
    global _nc_cache
    if _nc_cache is None:
        _nc_cache = _build()
    return _nc_cache


def run_cores(inputs, **kwargs):
    """Run the 8-core SPMD kernel; returns (loss[4], BassKernelResults)."""
    from concourse.bass_utils import run_bass_kernel_spmd

    nc = _get_nc()
    X = np.asarray(inputs["X_features"], dtype=np.float32).reshape(B, C, HW)
    Y = np.asarray(inputs["Y_features"], dtype=np.float32).reshape(B, C, HW)
    in_maps = []
    for core in range(NCORES):
        b, h = divmod(core, 2)
        in_maps.append(
            {
                "y": np.ascontiguousarray(Y[b]),
                "xh": np.ascontiguousarray(X[b, :, h * HALF : (h + 1) * HALF]),
            }
        )
    res = run_bass_kernel_spmd(nc, in_maps, core_ids=list(range(NCORES)), **kwargs)
    acc = np.stack(
        [res.results[i]["out"].reshape(-1).astype(np.float64) for i in range(NCORES)]
    )  # [8, 128]
    cx = acc.reshape(B, 2 * 128).sum(axis=1) / HW
    loss = (-np.log(cx)).astype(np.float32)
    return loss, res


def kernel(**inputs):
    return run_cores(inputs)[0]
